# revision 9
# baseline (speedup 1.0000x reference)
"""Multi-head attention forward on 8 TRN2 NeuronCores — v2 (ACT/DVE-balanced).

Problem: x[2, 2048, 1024], 16 heads x 64 dims, nn.Linear-style Q/K/V/O
projections.  Core c owns batch c//4 and heads [4*(c%4), 4*(c%4)+4).

Engine plan (per core):
  ACT : ~72 of the 128 exp instructions + bcs broadcast copies + O copyback.
  DVE : ~56 exp tiles as single-op uint8-Schraudolph (f32->e4m3 bit pattern),
        Q/K flat quantize+bias (TSP), reciprocal, CT normalize mul, V8h copy.
  PE  : fp8 DoubleRow everywhere except O (bf16) and the K=1 bias/broadcast
        matmuls.  PV uses the V hi term only (pv1).
  DMA : Q/K partition-folds are single SBUF->SBUF DMAs — the host permutes
        weight columns so psum partition p = (q, h2, j) lexicographic, making
        the fold a contiguous linear-order copy.
Attention runs in 8 single-head sections; section j+1 interleaves section
j's PV stream.  Fillers (projection groups, o-tiles) drain on a per-section
budget as in v1.
"""

from contextlib import ExitStack

import ml_dtypes
import numpy as np

import concourse.bass as bass
import concourse.tile as tile
from concourse import mybir

BF16 = mybir.dt.bfloat16
F32 = mybir.dt.float32
F32R = mybir.dt.float32r
FP8 = mybir.dt.float8e4
U8 = mybir.dt.uint8
AF = mybir.ActivationFunctionType
DR = mybir.MatmulPerfMode.DoubleRow
NPF8 = ml_dtypes.float8_e4m3
NPBF16 = ml_dtypes.bfloat16

P = 128
B = 2
NTOK = 2048
ED = 1024
KD = 8               # 8 k-tiles of 128 over the 1024 contraction dim
NE = 2               # e-tiles per core (4 heads x 64 = 256 dims)
NH_CORE = 4
HD = 64
MT = 16              # key/value m-tiles of 128
NCHUNK = 1024        # query-token chunk (half the tokens)
VCOL = 96            # per-head V block: 64 dims + ones col + 31 zero pad
VROW = NH_CORE * VCOL
SW = 32.0            # host weight prescale
EXP_SCALE = 1.0 / (8.0 * SW * SW)   # 1/sqrt(64) / SW^2
SCH8_A = EXP_SCALE * 8.0 * float(np.log2(np.e))   # e4m3-bits Schraudolph
SCH8_B = 56.05
# per-section m-tiles handled by the DVE uint8-Schraudolph (rebalances the
# exp mass against each section's other DVE work)
SCH_MASKS = [
    (2, 6, 9, 12, 15),           # S0: DVE also runs prologue/K flats
    (4, 8, 12, 15),              # S1: DVE busy with V8h copies
    (2, 5, 8, 10, 13, 15),
    (2, 5, 8, 10, 13, 15),
    (2, 5, 8, 10, 13, 15),
    (2, 5, 8, 10, 13, 15),
    (2, 5, 8, 10, 13, 15),
    (2, 5, 8, 10, 13, 15),
]


def _ap(t, extra, dims):
    return bass.AP(tensor=t.tensor, offset=t.offset + extra, ap=dims)


def _mha_body(ctx: ExitStack, tc: tile.TileContext, outs: dict, ins: dict):
    nc = tc.nc
    x8h_d, x8l_d = ins["x8h"], ins["x8l"]            # [128, 8, 2048] fp8
    wq_d = [ins["wq0"], ins["wq1"]]                  # [128, 8, 2, 128] fp8
    wk_d = [ins["wk0"], ins["wk1"]]
    wv_d = ins["wv"]                                 # [128, 8, 2, 256] fp8
    bqk_d = ins["bqk"]                               # [128, 4] f32 (perm, *SW)
    bve_d = ins["bve"]                               # [1, 4, 65] bf16
    wo_d = ins["wo"]                                 # [128, 2, 1024] bf16
    out = outs["out"]                                # [2048, 1024] bf16

    ctx.enter_context(nc.allow_low_precision(reason="f32r norm broadcast"))
    const = ctx.enter_context(tc.tile_pool(name="const", bufs=1))
    sb_big = ctx.enter_context(tc.tile_pool(name="sb_big", bufs=1))
    sb_fl = ctx.enter_context(tc.tile_pool(name="sb_fl", bufs=8))
    sb_ex = ctx.enter_context(tc.tile_pool(name="sb_ex", bufs=12))
    sb_sm = ctx.enter_context(tc.tile_pool(name="sb_sm", bufs=4))
    sb_out = ctx.enter_context(tc.tile_pool(name="sb_out", bufs=24))
    psum = ctx.enter_context(tc.tile_pool(name="psum", bufs=1, space="PSUM"))

    # ---- SBUF residents
    x8h = sb_big.tile([P, KD, NTOK], FP8)
    x8l = sb_big.tile([P, KD, NTOK], FP8)
    wq_sb = [const.tile([P, KD, 2, P], FP8, name=f"wq{t}") for t in range(2)]
    wk_sb = [const.tile([P, KD, 2, P], FP8, name=f"wk{t}") for t in range(2)]
    wv_sb = const.tile([P, KD, 2, 2 * P], FP8)
    bqk = const.tile([P, 4], F32)
    bve = const.tile([1, NH_CORE, 65], BF16)
    wo_sb = const.tile([P, NE, ED], BF16)
    ones = const.tile([1, 512], BF16)
    QT8 = sb_big.tile([32, NH_CORE, 2, NTOK], FP8)
    KT8 = sb_big.tile([32, NH_CORE, 2, NTOK], FP8)
    V8h = sb_big.tile([P, MT, VROW], FP8)
    CT = sb_big.tile([P, NE, NTOK], BF16)
    ones_f = const.tile([1, HD], F32)
    ones_r = const.tile([1, HD], F32R)

    # ---- input DMA: ONLY what the prologue needs up front (the cost
    # model serializes the shared DMA device, so every early byte delays
    # the first exp).  e-tile-0 K/Q weights on SP; prologue x pieces on
    # the ACT HWDGE queue in the order the projection matmuls consume
    # them (x-hi first: the third term reads x-lo last).
    nc.sync.dma_start(wk_sb[0], wk_d[0])
    nc.sync.dma_start(bqk, bqk_d)
    nc.sync.dma_start(wq_sb[0], wq_d[0])
    for n_ in (0, 1):
        s_ = np.s_[:, :, n_ * 512:(n_ + 1) * 512]
        nc.scalar.dma_start(x8h[s_], x8h_d[s_])
    for n_ in (0, 1):
        s_ = np.s_[:, :, n_ * 512:(n_ + 1) * 512]
        nc.scalar.dma_start(x8l[s_], x8l_d[s_])
    nc.sync.dma_start(wv_sb, wv_d)
    nc.sync.dma_start(bve, bve_d)

    def _gate(dst_byte_ap, src_ap):
        # WAW fence: a 1-byte Pool copy that reads prologue data and dirties
        # the DMA destination, so the load can't start before the source
        # tile exists (the DMA then overwrites the byte)
        nc.gpsimd.tensor_copy(dst_byte_ap, src_ap)

    def late_loads(n):
        if n < 4:
            s_ = np.s_[:, :, n * 512:(n + 1) * 512]
            _gate(x8h[0:1, 0:1, n * 512:n * 512 + 1],
                  KT8[0:1, 0:1, 0:1, 0:1])
            nc.gpsimd.dma_start(x8h[s_], x8h_d[s_])
            _gate(x8l[0:1, 0:1, n * 512:n * 512 + 1],
                  KT8[0:1, 0:1, 0:1, 0:1])
            nc.gpsimd.dma_start(x8l[s_], x8l_d[s_])
        elif n == 4:
            _gate(wo_sb.bitcast(FP8)[0:1, 0:1, 0:1],
                  QT8[0:1, 0:1, 0:1, 512:513])
            nc.gpsimd.dma_start(wo_sb, wo_d)
        elif n == 5:
            pass
        else:
            nc.sync.dma_start(wq_sb[1], wq_d[1])
            nc.sync.dma_start(wk_sb[1], wk_d[1])
    nc.vector.memset(ones, 1.0)
    nc.vector.memset(ones_f, 1.0 / SW)   # folds V's 32-scale into 1/den
    nc.vector.tensor_copy(ones_r, ones_f)
    # zero the 31-col pads of each 96-block once (cols 65..95)
    pad = _ap(V8h, 65, [V8h.ap[0], [VROW, MT], [VCOL, NH_CORE], [1, 31]])
    nc.vector.memset(pad, 0.0)

    pj = [0]

    # warm the PE p-state ramp
    ps_w = psum.tile([64, 64], F32, tag="o0", name="ps_warm")
    nc.tensor.matmul(ps_w, ones[:, 0:64], ones[:, 0:64], start=True, stop=True)

    oslots = [2]     # projection psum slots: 3 while cx is unallocated (S0)

    def qk_group(w_sb, bc_, dst, t, n, q=None, act_flat=False, terms=3):
        """Q or K projection of e-tile t over tokens [512n, 512n+512):
        3-term fp8 DR; quantize+bias via one TSP; fold via one DMA (host
        weight permutation makes flat's partition order = dst layout order).
        Term order keeps the x-lo read last so prologue x-hi DMAs unblock
        the first eight matmuls."""
        w8t = w_sb[t]
        tg = ("o0", "o1", "cx")[pj[0] % oslots[0]]
        ps = psum.tile([P, 512], F32, tag=tg, name="ps_qk")
        pj[0] += 1
        ns_ = np.s_[n * 512:(n + 1) * 512]
        tl = ((0, x8h), (1, x8h), (0, x8l))[:terms]
        for ti, (hl, x8) in enumerate(tl):
            for kk in range(4):
                ks = np.s_[2 * kk:2 * kk + 2]
                nc.tensor.matmul(ps, w8t[:, ks, hl, :], x8[:, ks, ns_],
                                 start=(ti == 0 and kk == 0),
                                 stop=(ti == len(tl) - 1 and kk == 3),
                                 perf_mode=DR)
        flat = sb_fl.tile([P, 512], FP8, name="flat")
        if act_flat:
            # prologue: ACT is idle and Identity shares Exp's table
            nc.scalar.activation(flat, ps, AF.Identity, bias=bc_[:, t:t + 1])
        else:
            nc.vector.tensor_scalar(flat, ps, bc_[:, t:t + 1], None,
                                    mybir.AluOpType.add)
        (q or nc.sync).dma_start(dst[:, 2 * t:2 * t + 2, :, ns_], flat)

    def v_group(tt):
        """V projection of token-tile tt (128 tokens) directly as
        [tok, 4 x 96-block] incl ones col; writes V8h (no lo term)."""
        tg = ("o0", "o1", "cx")[pj[0] % oslots[0]]
        ps = psum.tile([P, 512], F32, tag=tg, name="ps_v")
        pj[0] += 1
        ts_ = np.s_[tt * P:(tt + 1) * P]
        vout = _ap(ps, 0, [ps.ap[0], [VCOL, NH_CORE], [1, HD]])
        for x8, hl in ((x8h, 0), (x8l, 0), (x8h, 1)):
            for kk in range(4):
                ks = np.s_[2 * kk:2 * kk + 2]
                nc.tensor.matmul(vout, x8[:, ks, ts_], wv_sb[:, ks, hl, :],
                                 start=(x8 is x8h and hl == 0 and kk == 0),
                                 stop=False, perf_mode=DR)
        vout_b = _ap(ps, 0, [ps.ap[0], [VCOL, NH_CORE], [1, 65]])
        nc.tensor.matmul(vout_b, ones[:, 0:P], bve, start=False, stop=True)
        hi = _ap(V8h, tt * VROW, [V8h.ap[0], [VCOL, NH_CORE], [1, 65]])
        nc.vector.tensor_copy(hi, vout_b)

    def o_tile(c, t, tail=False):
        """O-projection of 128 tokens x 1024 out-dims (bf16).  Copybacks on
        ACT; out-DMAs rotate across queues."""
        r = c * NCHUNK + t * P
        ob = sb_out.tile([P, ED], BF16, name="ob")
        for u in range(2):
            if tail:
                tg = ("o0", "o1", "s0", "s1")[pj[0] % 4]
            else:
                tg = f"o{pj[0] % 2}"
            po = psum.tile([P, 512], F32, tag=tg, name="ps_o")
            pj[0] += 1
            us_ = np.s_[u * 512:(u + 1) * 512]
            for k in range(NE):
                nc.tensor.matmul(po, CT[:, k, r:r + P], wo_sb[:, k, us_],
                                 start=(k == 0), stop=(k == NE - 1))
            if u == 1:
                nc.vector.tensor_copy(ob[:, us_], po)
            else:
                nc.scalar.copy(ob[:, us_], po)
        eng = (nc.sync, nc.scalar)[t % 2] if tail else nc.sync
        eng.dma_start(out[r:r + P, :], ob)

    def pv(st, tp):
        """PV pair tp of the carried section state st (V hi term only)."""
        ex = st["exs"][tp]
        off = 2 * tp * VROW + VCOL * st["h"]
        lhsT = _ap(V8h, off, [V8h.ap[0], [VROW, 2], [1, VCOL]])
        for u in range(2):
            nc.tensor.matmul(
                st["cx"][:, u * 512:(u + 1) * 512], lhsT,
                ex[:, :, u * 512:(u + 1) * 512],
                start=(tp == 0),
                stop=(tp == MT // 2 - 1), perf_mode=DR)

    def norm(st, halves=(0, 1), bc_tags=None, bcs_dve=False):
        """Normalize the carried section: CT = cx[0:64] / (32 * den).
        rcp on DVE, K=1 f32r broadcast matmul on PE, bcs copy on ACT,
        final mul on DVE."""
        c, h = st["c"], st["h"]
        te, hh = h // 2, h % 2
        g = c * NCHUNK
        cx = st["cx"]
        for v in halves:
            vs = np.s_[v * 512:(v + 1) * 512]
            rcp = sb_sm.tile([1, 512], F32R, tag=f"rcp{v}", name="rcp")
            nc.vector.reciprocal(rcp, cx[64:65, vs])
            tg_ = bc_tags[v] if bc_tags else f"o{pj[0] % 2}"
            bc = psum.tile([HD, 512], F32, tag=tg_, name="bc")
            pj[0] += 1
            nc.tensor.matmul(bc, ones_r, rcp, start=True, stop=True)
            bcs = sb_sm.tile([HD, 512], F32, tag=f"bc{v}", name="bcs")
            if bcs_dve:
                nc.vector.tensor_copy(bcs, bc)
            else:
                nc.scalar.copy(bcs, bc)
            nc.vector.tensor_mul(
                CT[64 * hh:64 * hh + HD, te, g + v * 512:g + v * 512 + 512],
                cx[0:HD, vs], bcs)

    def section(c, h, carry, fillers, budget, last=False, sch=(),
                bcs_dve=False, start_m=0, pre=None):
        """One (chunk, head) attention section: 16 QK+exp m-steps, the
        carried section's PV at odd m, fillers drained by budget.  m-tiles
        in sch quantize via the DVE uint8-Schraudolph instead of ACT exp."""
        g = c * NCHUNK
        st = {"c": c, "h": h, "exs": [], "cx": None}
        if pre is not None:
            st["exs"].append(pre)
        debt = [0.0]
        ex = None
        for m in range(start_m, MT):
            with tc.high_priority(offset=1 << 20):
                sc = psum.tile([P, NCHUNK], F32, tag=f"s{m % 2}", name="sc")
                ms_ = np.s_[m * P:(m + 1) * P]
                for u in range(2):
                    qs_ = np.s_[g + u * 512:g + u * 512 + 512]
                    nc.tensor.matmul(sc[:, u * 512:(u + 1) * 512],
                                     KT8[:, h, :, ms_], QT8[:, h, :, qs_],
                                     start=True, stop=True, perf_mode=DR)
                if m % 2 == 0:
                    ex = sb_ex.tile([P, 2, NCHUNK], FP8, name="ex")
                if m in sch:
                    nc.vector.tensor_scalar(ex[:, m % 2, :].bitcast(U8), sc,
                                            float(SCH8_A), float(SCH8_B),
                                            mybir.AluOpType.mult,
                                            mybir.AluOpType.add)
                else:
                    nc.scalar.activation(ex[:, m % 2, :], sc, AF.Exp,
                                         scale=EXP_SCALE)
            if m % 2 == 1:
                st["exs"].append(ex)
            if carry is not None:
                if carry["cx"] is None:
                    carry["cx"] = psum.tile([VCOL, NCHUNK], F32,
                                            tag="cx", name="cx")
                if last:
                    if m < MT // 2:
                        pv(carry, m)
                    elif m == MT // 2:
                        norm(carry, bcs_dve=bcs_dve)
                elif m % 2 == 1:
                    pv(carry, (m - 1) // 2)
            if last and m >= MT // 2 + 1:
                if st["cx"] is None:
                    st["cx"] = psum.tile([VCOL, NCHUNK], F32,
                                         tag="cx", name="cx_own")
                pv(st, m - (MT // 2 + 1))
            debt[0] += budget
            while fillers and debt[0] >= fillers[0][0]:
                w, fn = fillers.pop(0)
                debt[0] -= w
                fn()
        if carry is not None and not last:
            norm(carry, bcs_dve=bcs_dve)
        return st

    # ---- prologue: what section (0,0) m=0 needs (K first); folds on the
    # ACT HWDGE queue (SP still busy with weight loads)
    QG = (wq_sb, bqk[:, 0:2], QT8)
    KG = (wk_sb, bqk[:, 2:4], KT8)
    # 2-term prologue groups: the x-lo pieces leave the critical DMA path
    # (costs +1e-4 rel err; keys 0:512 / queries 0:1024 of e-tile 0 only)
    oslots[0] = 3
    qk_group(*KG, 0, 0, q=nc.scalar, act_flat=True, terms=2)
    qk_group(*QG, 0, 0, q=nc.scalar, act_flat=True, terms=2)
    qk_group(*QG, 0, 1, q=nc.scalar, act_flat=True, terms=2)

    # ---- fillers in deadline order; weights ~ their PE microseconds
    GW, VW, OW = 1.3, 0.76, 0.9
    F = []
    # S0: K(0,1..3) due m4/m8/m12, V0..V4 due early S1
    F += [(0.1, (late_loads, (5,))), (0.1, (late_loads, (2,))),
          (GW, (qk_group, (*KG, 0, 1))), (0.1, (late_loads, (3,))),
          (VW, (v_group, (0,))),
          (VW, (v_group, (1,))), (GW, (qk_group, (*KG, 0, 2))),
          (VW, (v_group, (2,))), (VW, (v_group, (3,))),
          (GW, (qk_group, (*KG, 0, 3))), (VW, (v_group, (4,)))]
    # S1: V5..V15 (pair p of S0's PV needs tiles <=2p+1), then Q(0,2/3)
    F += [(0.1, (late_loads, (6,)))]
    F += [(VW, (v_group, (tt,))) for tt in range(5, 16)]
    F += [(0.1, (late_loads, (4,)))]
    F += [(GW, (qk_group, (*QG, 0, 2))), (GW, (qk_group, (*QG, 0, 3)))]
    # S2: e-tile-1 K/Q due at S3=(0,2): K(1,0) m0, Q(1,0/1) m0, K(1,1) m4
    F += [(GW, (qk_group, (*KG, 1, 0))), (GW, (qk_group, (*QG, 1, 0))),
          (GW, (qk_group, (*QG, 1, 1))), (GW, (qk_group, (*KG, 1, 1)))]
    # S3: K(1,2) due m8, K(1,3) due m12
    F += [(GW, (qk_group, (*KG, 1, 2))), (GW, (qk_group, (*KG, 1, 3)))]
    # S4: Q(1,2/3) due at S6=(1,2)
    F += [(GW, (qk_group, (*QG, 1, 2))), (GW, (qk_group, (*QG, 1, 3)))]
    fillers = [(w, (lambda f=f, a=a: f(*a))) for w, (f, a) in F]

    # ---- 8 sections; section j+1 carries section j's PV stream.
    # chunk-0 heads at S0/S1/S3/S4 -> CT chunk 0 normed by end of S5,
    # so o(c0) tiles spread across S6 and S7.
    order = [(0, 0), (0, 1), (1, 0), (0, 2), (0, 3), (1, 1), (1, 2), (1, 3)]
    budgets = [0.70, 0.80, 0.40, 0.28, 0.28, 0.20, 0.42, 0.42]
    st = None
    for i, (c, h) in enumerate(order):
        if i == 1:
            oslots[0] = 2
        if i == 6:
            fillers.extend(
                (OW, (lambda t=t: o_tile(0, t))) for t in range(4))
        if i == 7:
            fillers.extend(
                (OW, (lambda t=t: o_tile(0, t))) for t in range(4, 8))
        st = section(c, h, st, fillers, budgets[i], last=(i == 7),
                     sch=SCH_MASKS[i])
    while fillers:
        fillers.pop(0)[1]()

    # ---- tail: last own PV pairs, then per-v-half norm + O(c1)
    for tp in range(MT - (MT // 2 + 1), MT // 2):
        pv(st, tp)
    # both norm halves are data-ready once pair 7 lands; emitting them
    # before the o-tiles keeps the v1 bcs copy ahead of the ob copies on
    # ACT so o(1, 4..7) aren't serialized behind o(1, 0..3).
    # The first two tiles' k=0 accumulation (heads 0-1, normed sections
    # ago) starts even before the norm — only k=1 stays norm-gated.
    pre = []
    for t in range(1):
        r = NCHUNK + t * P
        obt = sb_out.tile([P, ED], BF16, name="ob")
        pos = []
        for u in range(2):
            po = psum.tile([P, 512], F32, tag=f"o{u}", name="ps_o")
            pj[0] += 1
            nc.tensor.matmul(po, CT[:, 0, r:r + P],
                             wo_sb[:, 0, u * 512:(u + 1) * 512],
                             start=True, stop=False)
            pos.append(po)
        pre.append((r, obt, pos))
    norm(st, bc_tags=("s0", "s1"))
    for t, (r, obt, pos) in enumerate(pre):
        for u in range(2):
            us_ = np.s_[u * 512:(u + 1) * 512]
            nc.tensor.matmul(pos[u], CT[:, 1, r:r + P],
                             wo_sb[:, 1, us_], start=False, stop=True)
            if u == 1:
                nc.vector.tensor_copy(obt[:, us_], pos[u])
            else:
                nc.scalar.copy(obt[:, us_], pos[u])
        (nc.sync, nc.scalar)[t % 2].dma_start(out[r:r + P, :], obt)
    for t in range(1, 8):
        o_tile(1, t, tail=True)


def _split_multi_waits(nc):
    """This walrus build allows exactly one sync-wait per instruction
    (the ISA EVENTS field has a single slot).  Hoist extra waits into
    same-engine NoOps placed immediately before the instruction."""
    n = 0
    for f in nc.m.functions:
        for blk in f.blocks:
            out = []
            for inst in blk.instructions:
                si = getattr(inst, "sync_info", None)
                if si is not None and si.on_wait and len(si.on_wait) > 1:
                    waits = list(si.on_wait)
                    for w in waits[:-1]:
                        n += 1
                        out.append(mybir.InstNoOp(
                            name=f"I-wsplit-{n}",
                            engine=inst.engine,
                            ins=[], outs=[],
                            sync_info=mybir.SyncInfo(on_wait=[w], on_update=[]),
                        ))
                    si.on_wait = waits[-1:]
                out.append(inst)
            blk.instructions = out
    return n


_NC_CACHE = None


def _build_nc():
    global _NC_CACHE
    if _NC_CACHE is not None:
        return _NC_CACHE
    nc = bass.Bass("TRN2", target_bir_lowering=False, debug=False,
                   disable_frame_to_traceback=True)
    ins = {}
    for nm, shp, dt in (
            ("x8h", [P, KD, NTOK], FP8), ("x8l", [P, KD, NTOK], FP8),
            ("wq0", [P, KD, 2, P], FP8), ("wq1", [P, KD, 2, P], FP8),
            ("wk0", [P, KD, 2, P], FP8), ("wk1", [P, KD, 2, P], FP8),
            ("wv", [P, KD, 2, 2 * P], FP8),
            ("bqk", [P, 4], F32),
            ("bve", [1, NH_CORE, 65], BF16), ("wo", [P, NE, ED], BF16)):
        ins[nm] = nc.dram_tensor(nm, shp, dt, kind="ExternalInput").ap()
    outs = {
        "out": nc.dram_tensor("out", [NTOK, ED], BF16, kind="ExternalOutput").ap(),
    }
    with tile.TileContext(nc) as tc, ExitStack() as ctx:
        _mha_body(ctx, tc, outs, ins)
    _split_multi_waits(nc)
    # scrub source paths / caller frames from the BIR so the neuron compile
    # cache hits regardless of where kernel.py lives
    for f in nc.m.functions:
        for al in f.allocations:
            mls = getattr(al, "memorylocations", None)
            if mls:
                for ml in mls:
                    if getattr(ml, "ant_debug", None) is not None:
                        ml.ant_debug = None
        for blk in f.blocks:
            for inst in blk.instructions:
                if getattr(inst, "debug", None) is not None:
                    inst.debug = None
    _NC_CACHE = nc
    return nc


# permutation: psum partition p (within an e-tile's 128 dims) holds
# e-local dim  (p%4)//2*64 + (p%2)*32 + p//4   (h2, j, q lexicographic in p)
_PERM = np.array([(p % 4) // 2 * 64 + (p % 2) * 32 + p // 4
                  for p in range(P)], np.int64)


def _prep_w_qk(w_slice: np.ndarray):
    """32*w[e0+128t : e0+128t+128, :] -> [128, 8, 2, 128] fp8 hi/lo packed,
    rows permuted so the fold is a linear-order DMA."""
    w32 = np.float32(SW) * np.asarray(w_slice, np.float32)[_PERM]  # [128,1024]
    wt = np.ascontiguousarray(w32.T)                      # [1024, 128]
    wl = np.ascontiguousarray(
        wt.reshape(KD, P, P).transpose(1, 0, 2))          # [128, 8, 128]
    hi = wl.astype(NPF8)
    lo = (wl - hi.astype(np.float32)).astype(NPF8)
    return np.ascontiguousarray(np.stack([hi, lo], axis=2))  # [128,8,2,128]


def _prep_w_v(w_slice: np.ndarray):
    w32 = np.float32(SW) * np.asarray(w_slice, np.float32)
    wt = np.ascontiguousarray(w32.T)                      # [1024, 256]
    wl = np.ascontiguousarray(
        wt.reshape(KD, P, 2 * P).transpose(1, 0, 2))      # [128, 8, 256]
    hi = wl.astype(NPF8)
    lo = (wl - hi.astype(np.float32)).astype(NPF8)
    return np.ascontiguousarray(np.stack([hi, lo], axis=2))  # [128,8,2,256]


def make_in_maps(x, wq, bq, wk, bk, wv, bv, wo, bo):
    x = np.asarray(x, np.float32).reshape(B, NTOK, ED)
    xh_by_batch, xl_by_batch = [], []
    for b in range(B):
        xt = np.ascontiguousarray(x[b].T)                 # [1024, 2048]
        xl_ = np.ascontiguousarray(
            xt.reshape(KD, P, NTOK).transpose(1, 0, 2))   # [128, 8, 2048]
        hi = xl_.astype(NPF8)
        lo = (xl_ - hi.astype(np.float32)).astype(NPF8)
        xh_by_batch.append(hi)
        xl_by_batch.append(lo)
    wq, wk, wv = (np.asarray(w, np.float32) for w in (wq, wk, wv))
    bq, bk, bv = (np.asarray(b_, np.float32) for b_ in (bq, bk, bv))
    wo = np.asarray(wo, np.float32)
    in_maps = []
    for c in range(8):
        b = c // 4
        e0 = (c % 4) * 256
        es = np.s_[e0:e0 + 256]
        wq0 = _prep_w_qk(wq[e0:e0 + 128])
        wq1 = _prep_w_qk(wq[e0 + 128:e0 + 256])
        wk0 = _prep_w_qk(wk[e0:e0 + 128])
        wk1 = _prep_w_qk(wk[e0 + 128:e0 + 256])
        wv_p = _prep_w_v(wv[es])
        bqk = np.empty((P, 4), np.float32)
        for t in range(2):
            bqk[:, t] = SW * bq[e0 + 128 * t + _PERM]
            bqk[:, 2 + t] = SW * bk[e0 + 128 * t + _PERM]
        bve = np.zeros((1, NH_CORE, 65), np.float32)
        for h in range(NH_CORE):
            bve[0, h, 0:HD] = SW * bv[e0 + HD * h:e0 + HD * h + HD]
            bve[0, h, HD] = 1.0
        wo_sl = np.ascontiguousarray(wo[:, es].T)         # [256, 1024]
        in_maps.append({
            "x8h": xh_by_batch[b], "x8l": xl_by_batch[b],
            "wq0": wq0, "wq1": wq1, "wk0": wk0, "wk1": wk1,
            "wv": wv_p,
            "bqk": bqk,
            "bve": bve.astype(NPBF16),
            "wo": np.ascontiguousarray(
                wo_sl.reshape(NE, P, ED).transpose(1, 0, 2)).astype(NPBF16),
        })
    return in_maps


_FN_CACHE = None


def _build_fn(nc, n_cores=8):
    """Multi-core PJRT executor (cached jitted shard_map callable)."""
    import jax
    from jax.sharding import Mesh, PartitionSpec
    from jax.experimental.shard_map import shard_map
    import concourse.bass2jax as b2j
    from concourse import mybir

    b2j.install_neuronx_cc_hook()
    pname = nc.partition_id_tensor.name if nc.partition_id_tensor else None
    in_names, out_names, out_avals = [], [], []
    for alloc in nc.m.functions[0].allocations:
        if not isinstance(alloc, mybir.MemoryLocationSet):
            continue
        name = alloc.memorylocations[0].name
        if alloc.kind == "ExternalInput":
            if name != pname:
                in_names.append(name)
        elif alloc.kind == "ExternalOutput":
            out_names.append(name)
            out_avals.append(jax.core.ShapedArray(
                tuple(alloc.tensor_shape), mybir.dt.np(alloc.dtype)))
    n_params = len(in_names)
    all_in = list(in_names) + list(out_names)
    if pname is not None:
        all_in.append(pname)

    def _body(*args):
        ops = list(args)
        if pname is not None:
            ops.append(b2j.partition_id_tensor())
        return tuple(b2j._bass_exec_p.bind(
            *ops,
            out_avals=tuple(out_avals), in_names=tuple(all_in),
            out_names=tuple(out_names), lowering_input_output_aliases=(),
            sim_require_finite=True, sim_require_nnan=True, nc=nc))

    devices = jax.devices()[:n_cores]
    mesh = Mesh(np.asarray(devices), ("core",))
    specs = (PartitionSpec("core"),) * (n_params + len(out_names))
    fn = jax.jit(shard_map(_body, mesh=mesh, in_specs=specs,
                           out_specs=(PartitionSpec("core"),) * len(out_names),
                           check_rep=False))
    zeros = [np.zeros((n_cores * a.shape[0], *a.shape[1:]), a.dtype)
             for a in out_avals]
    return fn, in_names, zeros


def kernel(x, wq, bq, wk, bk, wv, bv, wo, bo, **_ignored):
    global _FN_CACHE
    nc = _build_nc()
    in_maps = make_in_maps(x, wq, bq, wk, bk, wv, bv, wo, bo)
    if _FN_CACHE is None:
        _FN_CACHE = _build_fn(nc)
    fn, in_names, zeros = _FN_CACHE
    concat_in = [np.concatenate([in_maps[c][n] for c in range(8)], axis=0)
                 for n in in_names]
    outs = fn(*concat_in, *zeros)
    o = np.asarray(outs[0]).astype(np.float32).reshape(8, NTOK, ED)
    bo = np.asarray(bo, np.float32)
    res = np.empty((B, NTOK, ED), np.float32)
    for b in range(B):
        res[b] = o[4 * b:4 * b + 4].sum(axis=0) + bo
    return res
